# revision 1
# baseline (speedup 1.0000x reference)
"""Trainium2 Bass kernel for nn_Attention_spd (dense transformer attention with
pairwise score bias `spd`, head-drop rescale, and output projection).

Reference computation (b=4, n=1024, dim=512, heads=8, dim_head=64):
    qkv = x @ w_qkv ; q,k,v = split
    dots = q @ k^T * scale + spd
    attn = softmax(dots) * (head_keep * H / sum(head_keep))
    out  = (attn @ v) @ w_out + b_out

Sharding across 8 NeuronCores: core c handles batch c//2 and heads
4*(c%2) .. 4*(c%2)+3 (data parallel on batch x tensor parallel on heads).
Each core computes a partial output projection over its 4 heads; the host
sums the two partials per batch (cheap 2-way reduce) and adds b_out.

Device-side choices:
  - q/k/v/proj matmuls in fp32r (fp32 rounded to 11 mantissa bits, full PE
    speed, ~1e-4 relative error); attention probabilities in bf16.
  - Attention computed transposed: dotsT[j,i] = k @ q^T so the exp'd scores
    are directly the [K=j, N=i] moving operand of attn@v.
  - v augmented with a ones column (M=65): the attn@v matmul also emits the
    softmax denominator (row 64 of the PSUM output).
  - softmax skips max-subtraction (logits ~N(0,2); exp safe in fp32) —
    mathematically identical.
  - exp(dots + spd) = exp(dots) * exp(spd); exp(spd) is precomputed on the
    host in bf16 (halves the dominant DMA stream), and the combine is a bf16
    DVE multiply (2x mode) instead of an f32 add.
  - Head-PAIR batching through 2-bank (128x1024) PSUM tiles: the two heads'
    dots share one PSUM tile so exp / multiply / normalize run as single
    wide ops (ACT is the pacing engine; this halves its per-op overhead).
  - scale folded into wq on host; head_keep rescale folded into w_out rows;
    softmax normalization broadcast via a K=1 fp32r matmul (gpsimd
    partition_broadcast mis-handles base!=0 APs on HW).
  - DMA instruction count minimized (each dma_start costs ~650ns of
    sequencer + shared-HWDGE time), ordered so the first attention phase's
    dependencies land first.
"""
import os
import sys

for _p in ("/opt/trn_rl_repo", os.path.expanduser("~/.axon_site/_ro/trn_rl_repo")):
    if os.path.isdir(_p) and _p not in sys.path:
        sys.path.insert(0, _p)

import numpy as np
import ml_dtypes

import concourse.bass as bass  # noqa: F401
import concourse.tile as tile
from concourse import bacc, mybir
from concourse.bass_utils import run_bass_kernel_spmd

P = 128
B, N, DIM = 4, 1024, 512
HEADS = 8
DIM_HEAD = 64
SCALE = DIM_HEAD ** -0.5
HL = 4          # heads per core (local)
F32 = mybir.dt.float32
F32R = mybir.dt.float32r
BF16 = mybir.dt.bfloat16
ADD = mybir.AluOpType.add
MULT = mybir.AluOpType.mult
EXP = mybir.ActivationFunctionType.Exp

VARIANT = "bf16mul"

_NC = {}


def build_nc(variant=VARIANT):
    """Build the SPMD Bass program (identical on all 8 cores)."""
    nc = bacc.Bacc("TRN2", target_bir_lowering=False, debug=False, num_devices=8)
    xT = nc.dram_tensor("xT", [DIM, N], F32R, kind="ExternalInput").ap()
    # [qm0 | km0 | v | qm1 | km1] so a small early DMA unblocks the first phase
    w3 = nc.dram_tensor("w3", [DIM, 3 * HL * DIM_HEAD], F32R, kind="ExternalInput").ap()
    wo = nc.dram_tensor("wo", [DIM_HEAD, HL, DIM], F32R, kind="ExternalInput").ap()
    # exp(spd) in bf16: [hp, ib, jj, jb, s, ii] — per (hp, ib) contiguous,
    # with the head pair's (s) tiles adjacent so one DVE op covers both
    spdT = nc.dram_tensor("spdT", [2, 2, P, 8, 2, 512], BF16, kind="ExternalInput").ap()
    y = nc.dram_tensor("y", [N, DIM], F32, kind="ExternalOutput").ap()

    from contextlib import ExitStack

    with tile.TileContext(nc) as tc, ExitStack() as ctx:
        const = ctx.enter_context(tc.tile_pool(name="const", bufs=1))
        sb = ctx.enter_context(tc.tile_pool(name="sb", bufs=1))
        spd_pool = ctx.enter_context(tc.tile_pool(name="spd", bufs=3))
        ex_pool = ctx.enter_context(tc.tile_pool(name="ex", bufs=3))
        pr_pool = ctx.enter_context(tc.tile_pool(name="pr", bufs=3))
        nrm_pool = ctx.enter_context(tc.tile_pool(name="nrm", bufs=2))
        ps = ctx.enter_context(tc.tile_pool(name="ps", bufs=2, space="PSUM"))
        ps4 = ctx.enter_context(tc.tile_pool(name="ps4", bufs=4, space="PSUM"))

        # ---- resident loads -------------------------------------------------
        xT_sb = sb.tile([P, 4, N], F32R)
        w3_sb = sb.tile([P, 4, 768], F32R, tag="w3")
        xT_r = xT.rearrange("(kb p) n -> p kb n", p=P)
        w3_r = w3.rearrange("(kb p) m -> p kb m", p=P)
        nc.sync.dma_start(xT_sb[:], xT_r[:])
        nc.sync.dma_start(w3_sb[:, :, 0:256], w3_r[:, :, 0:256])      # q/k m0
        nc.sync.dma_start(w3_sb[:, :, 256:512], w3_r[:, :, 256:512])  # v
        wo_sb = sb.tile([DIM_HEAD, HL, DIM], F32R, tag="wo")

        ones32 = const.tile([P, 1], F32)
        nc.vector.memset(ones32[:], 1.0)
        # ones row at partition 64: lhsT of the K=1 rowsum-reciprocal
        # broadcast matmul (both operands at partition 64 — HW-exact)
        ones65f = const.tile([65, DIM_HEAD], F32, tag="ones65f")
        nc.vector.memset(ones65f[:], 1.0)
        ones65 = const.tile([65, DIM_HEAD], F32R, tag="ones65")
        nc.vector.tensor_copy(ones65[:], ones65f[:])
        wrowf = const.tile([65, 512], F32, tag="wrowf")
        nc.vector.memset(wrowf[:], 1.0)
        wrow = const.tile([65, 512], F32R, tag="wrow")
        nc.vector.tensor_copy(wrow[64:65, :], wrowf[64:65, :])

        # PE warm-up during the initial DMA wait: the PE clock-gate (HAM)
        # starts throttled; ~3.5us of dummy matmuls bring it to full rate
        # before the qkv projections arrive
        warm = ps.tile([P, 1024], F32, tag="big", name="warm")
        for w in range(16):
            nc.tensor.matmul(warm[0:64, 0:512], ones65[64:65, :], wrow[64:65, :],
                             start=True, stop=True)

        # ---- qkv projections ------------------------------------------------
        qT_sb = sb.tile([P, 2, N], F32R, tag="qT")
        kT_sb = sb.tile([P, 2, N], F32R, tag="kT")
        v_aug = sb.tile([P, 8, HL * 65], BF16, tag="vaug")
        v_cols = v_aug[:].rearrange("p jb (h c) -> p jb h c", c=65)
        nc.vector.tensor_copy(
            v_cols[:, :, :, 64:65],
            ones32[:, None, :, None].to_broadcast((P, 8, HL, 1)),
        )

        def qk_proj(qk, dst, m):
            wofs = (512 if m else 0) + qk * 128
            pq = ps.tile([P, 1024], F32, tag="big", name=f"pq_{qk}_{m}")
            for nb in range(2):
                for kb in range(4):
                    nc.tensor.matmul(
                        pq[:, nb * 512:(nb + 1) * 512],
                        w3_sb[:, kb, wofs:wofs + 128],
                        xT_sb[:, kb, nb * 512:(nb + 1) * 512],
                        start=(kb == 0),
                        stop=(kb == 3),
                    )
            nc.scalar.copy(dst[:, m, :], pq[:])

        qk_proj(0, qT_sb, 0)
        qk_proj(1, kT_sb, 0)
        # v: narrow tiles on the 4-slot ring (keeps the wide ring free for
        # the first attention phase's dots)
        for jb in range(8):
            pv = ps4.tile([P, 512], F32, tag="po", name=f"pv_{jb}")
            for kb in range(4):
                nc.tensor.matmul(
                    pv[:, :256],
                    xT_sb[:, kb, jb * 128:(jb + 1) * 128],
                    w3_sb[:, kb, 256:512],
                    start=(kb == 0),
                    stop=(kb == 3),
                )
            nc.vector.tensor_copy(
                v_cols[:, jb, :, :64],
                pv[:, :256].rearrange("p (h c) -> p h c", c=64),
            )

        # first attention phase's spd transfers go on the DMA queue ahead of
        # the late weight loads and the m1 q/k projections
        st00 = spd_pool.tile([P, 8, 2, 512], BF16, tag="spd", name="spd_0_0")
        nc.sync.dma_start(st00[:, 0:4], spdT[0, 0, :, 0:4])
        nc.sync.dma_start(st00[:, 4:8], spdT[0, 0, :, 4:8])
        nc.sync.dma_start(w3_sb[:, :, 512:768], w3_r[:, :, 512:768])  # q/k m1
        nc.sync.dma_start(wo_sb[:], wo[:])

        # ---- attention ------------------------------------------------------
        # scaled attention output, transposed: [d, h, i] (d on partitions)
        scaled = sb.tile([DIM_HEAD, HL, N], F32R, tag="scaled")
        y_all = sb.tile([P, 8, 512], F32, tag="yall")

        def do_norm(po, hp, ib):
            # head-pair normalization: 1/rowsums -> broadcast via K=1
            # matmuls -> rescale into `scaled`
            rc = nrm_pool.tile([65, 1024], F32R, tag="rc", name=f"rc_{hp}_{ib}")
            with nc.allow_low_precision(reason="f32r recip is plenty for softmax denom"):
                for s in range(2):
                    nc.vector.reciprocal(rc[64:65, s * 512:(s + 1) * 512],
                                         po[s][64:65, :])
            pb = ps.tile([P, 1024], F32, tag="big", name=f"pb_{hp}_{ib}")
            for s in range(2):
                nc.tensor.matmul(pb[0:64, s * 512:(s + 1) * 512],
                                 ones65[64:65, :], rc[64:65, s * 512:(s + 1) * 512],
                                 start=True, stop=True)
            bc = nrm_pool.tile([64, 1024], F32, tag="bc", name=f"bc_{hp}_{ib}")
            nc.vector.tensor_copy(bc[:], pb[0:64, :])
            for s in range(2):
                nc.vector.tensor_tensor(
                    scaled[:, 2 * hp + s, ib * 512:(ib + 1) * 512],
                    po[s][0:64, :],
                    bc[:, s * 512:(s + 1) * 512],
                    MULT,
                )

        def proj(iop):
            # narrow tiles from the 4-slot ring (the wide ring keeps feeding
            # the dots/exp stream)
            for half in range(2):
                io = 2 * iop + half
                py = ps4.tile([P, 512], F32, tag="po", name=f"py_{io}")
                for h in range(HL):
                    nc.tensor.matmul(
                        py[:],
                        scaled[:, h, io * 128:(io + 1) * 128],
                        wo_sb[:, h, :],
                        start=(h == 0),
                        stop=(h == HL - 1),
                    )
                nc.vector.tensor_copy(y_all[:, io, :], py[:])
            # gpsimd/SWDGE queue: an output DMA waiting on its copy must not
            # block the spd stream on the SP HWDGE queue
            nc.gpsimd.dma_start(
                y[iop * 256:(iop + 1) * 256, :].rearrange("(half p) q -> p half q", p=P),
                y_all[:, 2 * iop:2 * iop + 2, :])

        prev = None
        for ib in range(2):          # i block of 512 (outer: frees proj early)
            for hp in range(2):      # head pair (local heads 2hp, 2hp+1)
                def m1_chunk(qk, dst):
                    wofs = 512 + qk * 128
                    for nb in range(2):
                        pq1 = ps4.tile([P, 512], F32, tag="po",
                                       name=f"pq1_{qk}_{nb}")
                        for kb in range(4):
                            nc.tensor.matmul(
                                pq1[:],
                                w3_sb[:, kb, wofs:wofs + 128],
                                xT_sb[:, kb, nb * 512:(nb + 1) * 512],
                                start=(kb == 0),
                                stop=(kb == 3),
                            )
                        nc.vector.tensor_copy(
                            dst[:, 1, nb * 512:(nb + 1) * 512], pq1[:])

                if ib == 0 and hp == 0:
                    st = st00
                else:
                    st = spd_pool.tile([P, 8, 2, 512], BF16, tag="spd",
                                       name=f"spd_{hp}_{ib}")
                    nc.sync.dma_start(st[:, 0:4], spdT[hp, ib, :, 0:4])
                    nc.sync.dma_start(st[:, 4:8], spdT[hp, ib, :, 4:8])
                po = [ps4.tile([128, 512], F32, tag="po", name=f"po_{hp}_{ib}_{s}")
                      for s in range(2)]
                for jb in range(8):
                    pd = ps.tile([P, 1024], F32, tag="big", name=f"pd_{hp}_{ib}_{jb}")
                    # the pair's dots back-to-back: disjoint K=64 row groups
                    # can overlap in the PE array
                    for s in range(2):
                        nc.tensor.matmul(
                            pd[:, s * 512:(s + 1) * 512],
                            kT_sb[64 * s:64 * s + 64, hp, jb * 128:(jb + 1) * 128],
                            qT_sb[64 * s:64 * s + 64, hp, ib * 512:(ib + 1) * 512],
                            start=True,
                            stop=True,
                        )
                    # one wide exp + one wide bf16 multiply for both heads
                    ex = ex_pool.tile([P, 1024], BF16, tag="ex", name=f"ex_{hp}_{ib}_{jb}")
                    nc.scalar.activation(ex[:], pd[:], EXP)
                    pr = pr_pool.tile([P, 1024], BF16, tag="pr", name=f"pr_{hp}_{ib}_{jb}")
                    nc.vector.tensor_tensor(
                        pr[:], ex[:],
                        st[:, jb].rearrange("p s i -> p (s i)"),
                        MULT,
                    )
                    for s in range(2):
                        h = 2 * hp + s
                        nc.tensor.matmul(
                            po[s][0:65, :],
                            v_aug[:, jb, h * 65:(h + 1) * 65],
                            pr[:, s * 512:(s + 1) * 512],
                            start=(jb == 0),
                            stop=(jb == 7),
                        )
                    # interleave previous-phase epilogue work into this
                    # phase's mid-stream PE slack instead of its boundary
                    if prev is not None:
                        if jb == 2:
                            do_norm(*prev)
                        if prev[1] == 1:      # prev phase completed its ib
                            if jb == 4:
                                proj(prev[2] * 2)
                            if jb == 6:
                                proj(prev[2] * 2 + 1)
                    if ib == 0 and hp == 0:
                        if jb == 4:
                            m1_chunk(0, qT_sb)
                        if jb == 6:
                            m1_chunk(1, kT_sb)
                prev = (po, hp, ib)

        # flush: last phase's normalization + remaining projections
        p_po, p_hp, p_ib = prev
        do_norm(p_po, p_hp, p_ib)
        proj(2)
        proj(3)

    nc.compile()
    return nc


def _get_nc(variant=VARIANT):
    if variant not in _NC:
        _NC[variant] = build_nc(variant)
    return _NC[variant]


def make_in_maps(x, spd, head_keep, w_qkv, w_out, variant=VARIANT):
    x = np.asarray(x, np.float32)
    spd = np.asarray(spd, np.float32)
    keep = np.asarray(head_keep, np.float32)
    w_qkv = np.asarray(w_qkv, np.float32)
    w_out = np.asarray(w_out, np.float32)
    cfac = keep * (HEADS / keep.sum())

    in_maps = []
    for c in range(8):
        bi, hh = divmod(c, 2)
        h0 = hh * HL
        hs = slice(h0 * DIM_HEAD, (h0 + HL) * DIM_HEAD)
        xT = np.ascontiguousarray(x[bi].T)
        q_cols = w_qkv[:, hs] * np.float32(SCALE)
        k_cols = w_qkv[:, DIM + h0 * DIM_HEAD:DIM + (h0 + HL) * DIM_HEAD]
        v_cols_h = w_qkv[:, 2 * DIM + h0 * DIM_HEAD:2 * DIM + (h0 + HL) * DIM_HEAD]
        w3 = np.ascontiguousarray(np.concatenate(
            [q_cols[:, :128], k_cols[:, :128], v_cols_h,
             q_cols[:, 128:], k_cols[:, 128:]],
            axis=1,
        ))
        wo_rows = w_out[hs, :] * np.repeat(cfac[h0:h0 + HL], DIM_HEAD)[:, None]
        wo = np.ascontiguousarray(wo_rows.reshape(HL, DIM_HEAD, DIM).transpose(1, 0, 2))
        sp = spd[bi, h0:h0 + HL]  # [HL, i, j] with h = 2*hp + s
        # [hp, s, ib, ii, jb, jj] -> [hp, ib, jj, jb, s, ii]
        spdT = sp.reshape(2, 2, 2, 512, 8, 128).transpose(0, 2, 5, 4, 1, 3)
        spdT = np.exp(spdT).astype(ml_dtypes.bfloat16)
        in_maps.append({"xT": xT, "w3": w3, "wo": wo, "spdT": np.ascontiguousarray(spdT)})
    return in_maps


def kernel(x, spd, head_keep, w_qkv, w_out, b_out):
    assert x.shape == (B, N, DIM) and spd.shape == (B, HEADS, N, N)
    nc = _get_nc()
    in_maps = make_in_maps(x, spd, head_keep, w_qkv, w_out)
    res = run_bass_kernel_spmd(nc, in_maps, core_ids=list(range(8)))
    out = np.empty((B, N, DIM), np.float32)
    for bi in range(B):
        out[bi] = res.results[2 * bi]["y"] + res.results[2 * bi + 1]["y"]
    out += np.asarray(b_out, np.float32)[None, None, :]
    return out



# revision 39
# speedup vs baseline: 1.2964x; 1.2964x over previous
"""Trainium2 Bass kernel for nn_Attention_spd (dense transformer attention with
pairwise score bias `spd`, head-drop rescale, and output projection).

Reference computation (b=4, n=1024, dim=512, heads=8, dim_head=64):
    qkv = x @ w_qkv ; q,k,v = split
    dots = q @ k^T * scale + spd
    attn = softmax(dots) * (head_keep * H / sum(head_keep))
    out  = (attn @ v) @ w_out + b_out

Sharding across 8 NeuronCores: core c handles batch c//2 and heads
4*(c%2) .. 4*(c%2)+3 (data parallel on batch x tensor parallel on heads).
Each core computes a partial output projection over its 4 heads; the host
sums the two partials per batch (cheap 2-way reduce) and adds b_out.

Device-side design (cost model: matmul = moving-cols * 0.42ns; vector ops =
free-size * cycle_t; exp runs only on ACT, which makes ACT the steady-state
pacer at ~8.3us per attention phase):
  - x / w_qkv / w_out shipped in bf16; q/k kept in f32r on-chip; attention
    probabilities in bf16.
  - Attention computed transposed: dotsT[j,i] = k @ q^T so the exp'd scores
    are directly the [K=j, N=i] moving operand of attn@v.
  - v augmented with a ones column (M=65): the attn@v matmul also emits the
    softmax denominator (row 64 of the PSUM output).
  - softmax skips max-subtraction (logits ~N(0,2); exp safe in fp32).
  - exp(dots + spd) = exp(dots) * exp(spd); exp(spd) precomputed on host in
    bf16; the combine is a bf16 multiply (2x mode) on DVE, except one tile
    per phase on the otherwise-idle Pool engine -- its attn@v matmuls are
    issued last (with the accumulation stop flag) so the slow Pool op stays
    off the PE critical path.
  - Head-PAIR batching through 2-bank (128x1024) PSUM tiles: one wide exp +
    one wide multiply per (head-pair, jb).
  - Normalization at the START of the following phase: reciprocal of the
    rowsum rows, per-s K=1 f32r matmuls broadcast them across partitions
    into ps4-ring tiles (the wide pd ring stays free for dots), then two DVE
    multiplies write `scaled` with the pair's heads STACKED on 128
    partitions (s=1 partition-shifted) so the output projection contracts
    K=128 per head pair (half the PE cost of K=64 per head).
  - scale folded into wq on host; head_keep rescale folded into w_out rows.
  - xT DMA'd in 4 K-chunks right behind the first w_qkv columns so the first
    projection matmul starts ~2.5us in; dummy matmuls + a dummy activation
    warm the PE p-state ramp and the ACT exp table during the DMA wait.
  - First phase's exp(spd) tiles + late weights ride the otherwise-unused
    scalar-engine HWDGE queue in parallel with the x/w_qkv stream on the
    sync queue; y rides the gpsimd SWDGE queue (no HOL blocking anywhere).
"""
import os
import sys

for _p in ("/opt/trn_rl_repo", os.path.expanduser("~/.axon_site/_ro/trn_rl_repo")):
    if os.path.isdir(_p) and _p not in sys.path:
        sys.path.insert(0, _p)

import numpy as np
import ml_dtypes

import concourse.bass as bass  # noqa: F401
import concourse.tile as tile
from concourse import bacc, mybir
from concourse.bass_utils import run_bass_kernel_spmd

P = 128
B, N, DIM = 4, 1024, 512
HEADS = 8
DIM_HEAD = 64
SCALE = DIM_HEAD ** -0.5
HL = 4          # heads per core (local)
F32 = mybir.dt.float32
F32R = mybir.dt.float32r
BF16 = mybir.dt.bfloat16
ADD = mybir.AluOpType.add
MULT = mybir.AluOpType.mult
EXP = mybir.ActivationFunctionType.Exp

POOL_JB = 5     # the one pr-multiply per phase that runs on Pool

VARIANT = "v2"

_NC = {}


def build_nc(variant=VARIANT):
    """Build the SPMD Bass program (identical on all 8 cores)."""
    nc = bacc.Bacc("TRN2", target_bir_lowering=False, debug=False, num_devices=8)
    xT = nc.dram_tensor("xT", [DIM, N], BF16, kind="ExternalInput").ap()
    # [qm0 | km0 | v | qm1 | km1] so a small early DMA unblocks the first phase
    w3 = nc.dram_tensor("w3", [DIM, 3 * HL * DIM_HEAD], BF16, kind="ExternalInput").ap()
    # head-pair stacked rows: wo2[s*64+d, hp, :] = w_out[(2hp+s)*64+d, :]
    wo = nc.dram_tensor("wo", [P, 2, DIM], BF16, kind="ExternalInput").ap()
    # exp(spd) in bf16: [hp, ib, jj, jb, s, ii]
    spdT = nc.dram_tensor("spdT", [2, 2, P, 8, 2, 512], BF16, kind="ExternalInput").ap()
    y = nc.dram_tensor("y", [N, DIM], F32, kind="ExternalOutput").ap()

    from contextlib import ExitStack

    with tile.TileContext(nc) as tc, ExitStack() as ctx:
        const = ctx.enter_context(tc.tile_pool(name="const", bufs=1))
        sb = ctx.enter_context(tc.tile_pool(name="sb", bufs=1))
        spd_pool = ctx.enter_context(tc.tile_pool(name="spd", bufs=3))
        ex_pool = ctx.enter_context(tc.tile_pool(name="ex", bufs=5))
        pr_pool = ctx.enter_context(tc.tile_pool(name="pr", bufs=6))
        nrm_pool = ctx.enter_context(tc.tile_pool(name="nrm", bufs=2))
        ps = ctx.enter_context(tc.tile_pool(name="ps", bufs=2, space="PSUM"))
        ps4 = ctx.enter_context(tc.tile_pool(name="ps4", bufs=4, space="PSUM"))

        # ---- resident loads -------------------------------------------------
        # sync queue: w_qkv m0 cols, xT in K-chunks, v cols, per-phase spd
        xT_sb = sb.tile([P, 4, N], BF16)
        w3_sb = sb.tile([P, 4, 768], BF16, tag="w3")
        xT_r = xT.rearrange("(kb p) n -> p kb n", p=P)
        w3_r = w3.rearrange("(kb p) m -> p kb m", p=P)
        nc.sync.dma_start(w3_sb[:, :, 0:256], w3_r[:, :, 0:256])      # q/k m0
        for kb in range(4):
            nc.sync.dma_start(xT_sb[:, kb, :], xT_r[:, kb, :])
        nc.sync.dma_start(w3_sb[:, :, 256:512], w3_r[:, :, 256:512])  # v
        # one queue, dependency order: the DMA engines serve queue heads
        # fairly, so anything on a second queue would steal bandwidth from
        # the prologue-critical stream above
        st00 = spd_pool.tile([P, 8, 2, 512], BF16, tag="spd", name="spd_0_0")
        nc.sync.dma_start(st00[:, 0:4], spdT[0, 0, :, 0:4])
        nc.sync.dma_start(st00[:, 4:8], spdT[0, 0, :, 4:8])
        nc.sync.dma_start(w3_sb[:, :, 512:768], w3_r[:, :, 512:768])  # q/k m1
        wo_sb = sb.tile([P, 2, DIM], BF16, tag="wo")
        nc.sync.dma_start(wo_sb[:], wo[:])

        # PE p-state warm-up + ACT exp-table preload during the initial DMA
        # wait: the ramp clock starts at the first matmul and reaches full
        # rate 3us later, so start it as early as possible — a Pool memset
        # (not DVE, whose queue is behind other init work) feeds the first
        # dummy matmul at ~0.3us.
        wdat = const.tile([65, 512], BF16, tag="wdat")
        nc.vector.memset(wdat[:], 1.0)
        warm = ps.tile([P, 1024], F32, tag="big", name="warm")
        for w in range(2):
            nc.tensor.matmul(warm[0:64, 0:512], wdat[64:65, 0:64], wdat[64:65, :],
                             start=True, stop=True)
        warm_ex = const.tile([1, 8], BF16, tag="warm_ex")
        nc.scalar.activation(warm_ex[:], wdat[64:65, 0:8], EXP)

        ones32 = const.tile([P, 1], F32)
        nc.vector.memset(ones32[:], 1.0)
        # ones row at partition 64: lhsT of the K=1 rowsum-reciprocal
        # broadcast matmul (both operands at partition 64 — HW-exact)
        ones65f = const.tile([65, DIM_HEAD], F32, tag="ones65f")
        nc.vector.memset(ones65f[:], 1.0)
        ones65 = const.tile([65, DIM_HEAD], F32R, tag="ones65")
        nc.vector.tensor_copy(ones65[:], ones65f[:])

        # ---- qkv projections ------------------------------------------------
        qT_sb = sb.tile([P, 2, N], F32R, tag="qT")
        kT_sb = sb.tile([P, 2, N], F32R, tag="kT")
        v_aug = sb.tile([P, 8, HL * 65], BF16, tag="vaug")
        v_cols = v_aug[:].rearrange("p jb (h c) -> p jb h c", c=65)
        nc.vector.tensor_copy(
            v_cols[:, :, :, 64:65],
            ones32[:, None, :, None].to_broadcast((P, 8, HL, 1)),
        )

        # m0 q and k interleaved per K-chunk so the matmuls pipeline behind
        # the xT chunk DMAs; copies split in halves on ACT — the ib=0 halves
        # (cols 0:512) unblock the first dots, the others only gate jb4+
        pq0 = ps.tile([P, 1024], F32, tag="big", name="pq0")
        pk0 = ps.tile([P, 1024], F32, tag="big", name="pk0")
        for kb in range(4):
            for qk, pq in ((0, pq0), (1, pk0)):
                for nb in range(2):
                    nc.tensor.matmul(
                        pq[:, nb * 512:(nb + 1) * 512],
                        w3_sb[:, kb, qk * 128:qk * 128 + 128],
                        xT_sb[:, kb, nb * 512:(nb + 1) * 512],
                        start=(kb == 0),
                        stop=(kb == 3),
                    )
        nc.scalar.copy(qT_sb[:, 0, 0:512], pq0[:, 0:512])
        nc.scalar.copy(kT_sb[:, 0, 0:512], pk0[:, 0:512])
        # late halves on DVE (parallel with the ACT halves): they must not
        # trail into phase 0 or the pd ring would wait on them behind exp
        nc.vector.tensor_copy(qT_sb[:, 0, 512:1024], pq0[:, 512:1024])
        nc.vector.tensor_copy(kT_sb[:, 0, 512:1024], pk0[:, 512:1024])

        def v_proj():
            # v projection: narrow tiles on the 4-slot ring; emitted INSIDE
            # phase 0 after its jb0-3 dots so the exp stream starts ~2us
            # earlier (v is only needed by attn@v, a phase-length later)
            for jb in range(8):
                pv = ps4.tile([P, 512], F32, tag="po", name=f"pv_{jb}")
                for kb in range(4):
                    nc.tensor.matmul(
                        pv[:, :256],
                        xT_sb[:, kb, jb * 128:(jb + 1) * 128],
                        w3_sb[:, kb, 256:512],
                        start=(kb == 0),
                        stop=(kb == 3),
                    )
                nc.vector.tensor_copy(
                    v_cols[:, jb, :, :64],
                    pv[:, :256].rearrange("p (h c) -> p h c", c=64),
                )

        # ---- attention ------------------------------------------------------
        # normalized attention output, transposed, head-pair stacked:
        # scaled[s*64+d, hp, i] (128 partitions -> K=128 proj matmuls)
        scaled = sb.tile([P, 2, N], BF16, tag="scaled")
        y_all = sb.tile([P, 8, 512], F32, tag="yall")

        def do_norm(po, hp, ib, halves=1, flush=False):
            # head-pair normalization: 1/rowsums (DVE) -> the exp'd scores
            # copied out of PSUM head-pair-stacked (DVE can only read ONE
            # PSUM operand per op, so the rescale multiplicand must be in
            # SBUF; the copy also frees the po ring slots early) -> K=1
            # broadcast matmuls (s=1 written at partition base 64) ->
            # rescale into `scaled`. halves=2 splits the rescale along i so
            # the flush projections start earlier; flush=True borrows the
            # then-idle ACT engine for the PSUM copies.
            rc = nrm_pool.tile([65, 1024], F32R, tag="rc", name=f"rc_{hp}_{ib}")
            with nc.allow_low_precision(reason="f32r recip is plenty for softmax denom"):
                for s in range(2):
                    nc.vector.reciprocal(rc[64:65, s * 512:(s + 1) * 512],
                                         po[s][64:65, :])
            pocp = nrm_pool.tile([P, 512], BF16, tag="pocp", name=f"pocp_{hp}_{ib}")
            pbs = []
            for s in range(2):
                cp = nc.scalar.copy if flush else nc.vector.tensor_copy
                cp(pocp[s * 64:(s + 1) * 64, :], po[s][0:64, :])
                pb = ps4.tile([P, 512], F32, tag="po", name=f"pb_{hp}_{ib}_{s}")
                nc.tensor.matmul(pb[0:64, :], ones65[64:65, :],
                                 rc[64:65, s * 512:(s + 1) * 512],
                                 start=True, stop=True)
                pbs.append(pb)
            w = 512 // halves
            for h in range(halves):
                for s in range(2):
                    nc.vector.tensor_tensor(
                        scaled[s * 64:(s + 1) * 64, hp,
                               ib * 512 + h * w:ib * 512 + (h + 1) * w],
                        pocp[s * 64:(s + 1) * 64, h * w:(h + 1) * w],
                        pbs[s][0:64, h * w:(h + 1) * w],
                        MULT,
                    )

        def proj(io, copy_eng="dve", dma_eng="sync"):
            # one 128-row block of y: K=128 over each head pair, copy out,
            # stream to DRAM (engines chosen by the caller for tail overlap)
            py = ps4.tile([P, 512], F32, tag="po", name=f"py_{io}")
            for hp in range(2):
                nc.tensor.matmul(
                    py[:],
                    scaled[:, hp, io * 128:(io + 1) * 128],
                    wo_sb[:, hp, :],
                    start=(hp == 0),
                    stop=(hp == 1),
                )
            if copy_eng == "act":
                nc.scalar.copy(y_all[:, io, :], py[:])
            else:
                nc.vector.tensor_copy(y_all[:, io, :], py[:])
            getattr(nc, dma_eng).dma_start(
                y[io * 128:(io + 1) * 128, :], y_all[:, io, :])

        def m1_half(qk, nb, dst):
            # one [128, 512] quarter of the m1 q/k projection: 4 matmuls +
            # a DVE copy, small enough to slot between two dots pairs
            wofs = 512 + qk * 128
            pq1 = ps4.tile([P, 512], F32, tag="po", name=f"pq1_{qk}_{nb}")
            for kb in range(4):
                nc.tensor.matmul(
                    pq1[:],
                    w3_sb[:, kb, wofs:wofs + 128],
                    xT_sb[:, kb, nb * 512:(nb + 1) * 512],
                    start=(kb == 0),
                    stop=(kb == 3),
                )
            nc.vector.tensor_copy(
                dst[:, 1, nb * 512:(nb + 1) * 512], pq1[:])

        # ---- software-pipelined phase loop ----------------------------------
        # ACT paces the steady state (8 back-to-back 1us exps per phase); all
        # other work is scheduled around keeping its pd ping-pong fed:
        #   * dots for jb0..3 are issued ahead of everything else at a phase
        #     boundary (the next exp must never wait),
        #   * the late attn@v matmuls of a phase (jb 3, 6, 7 — Pool's jb3
        #     multiply and the last DVE multiplies finish near the boundary)
        #     are CARRIED into the next phase and issued s-major with the
        #     accumulation stop flags so the rowsum reciprocals start early,
        #   * the previous phase's normalization (recips -> K=1 broadcasts ->
        #     partition-shifted rescales) then runs in the next phase's PE/DVE
        #     slack, freeing its po slots just before attn@v jb0 needs them,
        #   * jb2/jb3 multiplies run on Pool (early, so their 2.2us latency
        #     hides inside the phase), the rest on DVE.
        def dots_exp_mult(hp, ib, jb, st, split_s=False, pool=False):
            pd = ps.tile([P, 1024], F32, tag="big", name=f"pd_{hp}_{ib}_{jb}")
            for s in range(2):
                nc.tensor.matmul(
                    pd[:, s * 512:(s + 1) * 512],
                    kT_sb[64 * s:64 * s + 64, hp, jb * 128:(jb + 1) * 128],
                    qT_sb[64 * s:64 * s + 64, hp, ib * 512:(ib + 1) * 512],
                    start=True,
                    stop=True,
                )
            ex = ex_pool.tile([P, 1024], BF16, tag="ex", name=f"ex_{hp}_{ib}_{jb}")
            pr = pr_pool.tile([P, 1024], BF16, tag="pr", name=f"pr_{hp}_{ib}_{jb}")
            eng = nc.gpsimd if pool else nc.vector
            if split_s:
                # per-s exp/multiply so head s=0's rowsum (and reciprocal)
                # is ready a half-tile earlier in the flush
                for s in range(2):
                    nc.scalar.activation(ex[:, s * 512:(s + 1) * 512],
                                         pd[:, s * 512:(s + 1) * 512], EXP)
                    eng.tensor_tensor(
                        pr[:, s * 512:(s + 1) * 512],
                        ex[:, s * 512:(s + 1) * 512],
                        st[:, jb, s],
                        MULT,
                    )
            else:
                nc.scalar.activation(ex[:], pd[:], EXP)
                eng.tensor_tensor(
                    pr[:], ex[:],
                    st[:, jb].rearrange("p s i -> p (s i)"),
                    MULT,
                )
            return pr

        def attnv(po, hp, jb, pr, start=False, stop=False, s_only=None):
            for s in range(2) if s_only is None else (s_only,):
                h = 2 * hp + s
                nc.tensor.matmul(
                    po[s][0:65, :],
                    v_aug[:, jb, h * 65:(h + 1) * 65],
                    pr[:, s * 512:(s + 1) * 512],
                    start=start,
                    stop=stop,
                )

        def carry_s(po, hp, carried, s):
            # one head's carried attn@v chain; the stop-flag matmul issues
            # as early as possible so its rowsum reciprocal overlaps the
            # other head's chain
            for n, (jb, pr) in enumerate(carried):
                attnv(po, hp, jb, pr, stop=(n == len(carried) - 1), s_only=s)

        prev = None      # (po, hp, ib, carried) of the previous phase
        for ib in range(2):          # i block of 512 (outer: frees proj early)
            for hp in range(2):      # head pair (local heads 2hp, 2hp+1)
                ph0 = (ib == 0 and hp == 0)
                last = (ib == 1 and hp == 1)
                if ph0:
                    st = st00
                else:
                    st = spd_pool.tile([P, 8, 2, 512], BF16, tag="spd",
                                       name=f"spd_{hp}_{ib}")
                    nc.sync.dma_start(st[:, 0:4], spdT[hp, ib, :, 0:4])
                    nc.sync.dma_start(st[:, 4:8], spdT[hp, ib, :, 4:8])
                pool_jbs = (2, 3)
                prs = {}
                prs[0] = dots_exp_mult(hp, ib, 0, st)
                prs[1] = dots_exp_mult(hp, ib, 1, st)
                # previous phase's late attn@v + normalization, split around
                # this phase's jb2/jb3 dots so the exp stream never starves
                if prev is not None:
                    carry_s(prev[0], prev[1], prev[3], 0)
                prs[2] = dots_exp_mult(hp, ib, 2, st, pool=2 in pool_jbs)
                if prev is not None:
                    carry_s(prev[0], prev[1], prev[3], 1)
                prs[3] = dots_exp_mult(hp, ib, 3, st, pool=3 in pool_jbs)
                if ib == 1 and hp == 0:
                    # qT m1 second half: needed by the (ib1, hp1) phase
                    m1_half(0, 1, qT_sb)
                if prev is not None:
                    do_norm(prev[0], prev[1], prev[2])
                if ph0:
                    # v projection rides the early-phase pd-ring slack (only
                    # attn@v consumes it, nearly a phase later)
                    v_proj()
                prs[4] = dots_exp_mult(hp, ib, 4, st, pool=4 in pool_jbs)
                if ph0:
                    m1_half(0, 0, qT_sb)
                prs[5] = dots_exp_mult(hp, ib, 5, st)
                if ph0:
                    m1_half(1, 0, kT_sb)
                if not ph0:
                    po = [ps4.tile([128, 512], F32, tag="po",
                                   name=f"po_{hp}_{ib}_{s}") for s in range(2)]
                    attnv(po, hp, 0, prs[0], start=True)
                    attnv(po, hp, 1, prs[1])
                prs[6] = dots_exp_mult(hp, ib, 6, st)
                prs[7] = dots_exp_mult(hp, ib, 7, st, split_s=last)
                if not ph0:
                    attnv(po, hp, 2, prs[2])
                    attnv(po, hp, 4, prs[4])
                if ph0:
                    # kT m1 second half at the very end of phase 0: its PE
                    # slot is free here and its DVE copy completes well
                    # before phase 1's jb4 dots need it
                    m1_half(1, 1, kT_sb)
                    # po allocated only after the m1 tiles: a ps4-ring slot
                    # reuse of po by an m1 tile would deadlock phase 1's dots
                    # against phase 1's own normalization
                    po = [ps4.tile([128, 512], F32, tag="po",
                                   name=f"po_{hp}_{ib}_{s}") for s in range(2)]
                    attnv(po, hp, 0, prs[0], start=True)
                    attnv(po, hp, 1, prs[1])
                else:
                    attnv(po, hp, 5, prs[5])
                if ib == 1 and not last:
                    proj(2 * hp + 0)
                    proj(2 * hp + 1)
                if ib == 1 and last:
                    # last phase: both projections after the final multiply,
                    # copies on the then-idle ACT
                    proj(2 * hp + 0, "act")
                    proj(2 * hp + 1, "act")
                carry = [2, 4, 5, 3, 6, 7] if ph0 else [3, 6, 7]
                prev = (po, hp, ib, [(jb, prs[jb]) for jb in carry])

        # flush: last phase's late attn@v + normalization + remaining
        # projections, spread across engines and all three DMA queues
        p_po, p_hp, p_ib, p_carried = prev
        carry_s(p_po, p_hp, p_carried, 0)
        carry_s(p_po, p_hp, p_carried, 1)
        do_norm(p_po, p_hp, p_ib, halves=2, flush=True)
        proj(4, "act", "gpsimd")
        proj(5, "dve", "sync")
        proj(6, "act", "scalar")
        proj(7, "dve", "gpsimd")

    nc.compile()
    return nc


def _get_nc(variant=VARIANT):
    if variant not in _NC:
        _NC[variant] = build_nc(variant)
    return _NC[variant]


def make_in_maps(x, spd, head_keep, w_qkv, w_out, variant=VARIANT):
    x = np.asarray(x, np.float32)
    spd = np.asarray(spd, np.float32)
    keep = np.asarray(head_keep, np.float32)
    w_qkv = np.asarray(w_qkv, np.float32)
    w_out = np.asarray(w_out, np.float32)
    cfac = keep * (HEADS / keep.sum())

    in_maps = []
    for c in range(8):
        bi, hh = divmod(c, 2)
        h0 = hh * HL
        hs = slice(h0 * DIM_HEAD, (h0 + HL) * DIM_HEAD)
        xT = np.ascontiguousarray(x[bi].T).astype(ml_dtypes.bfloat16)
        q_cols = w_qkv[:, hs] * np.float32(SCALE)
        k_cols = w_qkv[:, DIM + h0 * DIM_HEAD:DIM + (h0 + HL) * DIM_HEAD]
        v_cols_h = w_qkv[:, 2 * DIM + h0 * DIM_HEAD:2 * DIM + (h0 + HL) * DIM_HEAD]
        w3 = np.ascontiguousarray(np.concatenate(
            [q_cols[:, :128], k_cols[:, :128], v_cols_h,
             q_cols[:, 128:], k_cols[:, 128:]],
            axis=1,
        )).astype(ml_dtypes.bfloat16)
        wo_rows = w_out[hs, :] * np.repeat(cfac[h0:h0 + HL], DIM_HEAD)[:, None]
        # [h, d, dim] -> [hp, s, d, dim] -> [(s d), hp, dim]
        wo2 = wo_rows.reshape(2, 2, DIM_HEAD, DIM).transpose(1, 2, 0, 3).reshape(
            P, 2, DIM)
        wo2 = np.ascontiguousarray(wo2).astype(ml_dtypes.bfloat16)
        sp = spd[bi, h0:h0 + HL]  # [HL, i, j] with h = 2*hp + s
        # [hp, s, ib, ii, jb, jj] -> [hp, ib, jj, jb, s, ii]
        spdT = sp.reshape(2, 2, 2, 512, 8, 128).transpose(0, 2, 5, 4, 1, 3)
        spdT = np.exp(spdT).astype(ml_dtypes.bfloat16)
        in_maps.append({"xT": xT, "w3": w3, "wo": wo2,
                        "spdT": np.ascontiguousarray(spdT)})
    return in_maps


def kernel(x, spd, head_keep, w_qkv, w_out, b_out):
    assert x.shape == (B, N, DIM) and spd.shape == (B, HEADS, N, N)
    nc = _get_nc()
    in_maps = make_in_maps(x, spd, head_keep, w_qkv, w_out)
    res = run_bass_kernel_spmd(nc, in_maps, core_ids=list(range(8)))
    out = np.empty((B, N, DIM), np.float32)
    for bi in range(B):
        out[bi] = res.results[2 * bi]["y"] + res.results[2 * bi + 1]["y"]
    out += np.asarray(b_out, np.float32)[None, None, :]
    return out


# revision 49
# speedup vs baseline: 1.3008x; 1.0034x over previous
"""Trainium2 Bass kernel for nn_Attention_spd (dense transformer attention with
pairwise score bias `spd`, head-drop rescale, and output projection).

Reference computation (b=4, n=1024, dim=512, heads=8, dim_head=64):
    qkv = x @ w_qkv ; q,k,v = split
    dots = q @ k^T * scale + spd
    attn = softmax(dots) * (head_keep * H / sum(head_keep))
    out  = (attn @ v) @ w_out + b_out

Sharding across 8 NeuronCores: core c handles batch c//2 and heads
4*(c%2) .. 4*(c%2)+3 (data parallel on batch x tensor parallel on heads).
Each core computes a partial output projection over its 4 heads; the host
sums the two partials per batch (cheap 2-way reduce) and adds b_out.

Device-side design (cost model: matmul = moving-cols * 0.42ns; vector ops =
free-size * cycle_t; exp runs only on ACT, which makes ACT the steady-state
pacer at ~8.3us per attention phase):
  - x / w_qkv / w_out shipped in bf16; q/k kept in f32r on-chip; attention
    probabilities in bf16.
  - Attention computed transposed: dotsT[j,i] = k @ q^T so the exp'd scores
    are directly the [K=j, N=i] moving operand of attn@v.
  - v augmented with a ones column (M=65): the attn@v matmul also emits the
    softmax denominator (row 64 of the PSUM output).
  - softmax skips max-subtraction (logits ~N(0,2); exp safe in fp32).
  - exp(dots + spd) = exp(dots) * exp(spd); exp(spd) precomputed on host in
    bf16; the combine is a bf16 multiply (2x mode) on DVE, except one tile
    per phase on the otherwise-idle Pool engine -- its attn@v matmuls are
    issued last (with the accumulation stop flag) so the slow Pool op stays
    off the PE critical path.
  - Head-PAIR batching through 2-bank (128x1024) PSUM tiles: one wide exp +
    one wide multiply per (head-pair, jb).
  - Normalization carried into the following phase: reciprocals of the
    rowsum rows; the exp'd sums are copied out of PSUM head-pair-stacked
    (DVE reads at most one PSUM operand per op, and the copy frees the po
    ring early); per-s K=1 f32r matmuls broadcast the reciprocals across
    partitions into ps4-ring tiles (the wide pd ring stays free for dots);
    two DVE multiplies then write `scaled` (bf16, s=1 partition-shifted to
    rows 64:128) so the output projection contracts K=128 per head pair
    (half the PE cost of K=64 per head).
  - scale folded into wq on host; head_keep rescale folded into w_out rows.
  - xT DMA'd in 4 K-chunks right behind the first w_qkv columns so the first
    projection matmul starts ~2.5us in; dummy matmuls + a dummy activation
    warm the PE p-state ramp and the ACT exp table during the DMA wait.
  - All input loads share the sync queue in dependency order (the DMA
    engines serve queue heads fairly, so a second queue would steal
    bandwidth from the prologue-critical stream); y rides the sync/gpsimd/
    scalar queues so no output DMA head-of-line-blocks the spd stream.
"""
import os
import sys

for _p in ("/opt/trn_rl_repo", os.path.expanduser("~/.axon_site/_ro/trn_rl_repo")):
    if os.path.isdir(_p) and _p not in sys.path:
        sys.path.insert(0, _p)

import numpy as np
import ml_dtypes

import concourse.bass as bass  # noqa: F401
import concourse.tile as tile
from concourse import bacc, mybir
from concourse.bass_utils import run_bass_kernel_spmd

P = 128
B, N, DIM = 4, 1024, 512
HEADS = 8
DIM_HEAD = 64
SCALE = DIM_HEAD ** -0.5
HL = 4          # heads per core (local)
F32 = mybir.dt.float32
F32R = mybir.dt.float32r
BF16 = mybir.dt.bfloat16
ADD = mybir.AluOpType.add
MULT = mybir.AluOpType.mult
EXP = mybir.ActivationFunctionType.Exp

POOL_JB = 5     # the one pr-multiply per phase that runs on Pool

VARIANT = "v2"

_NC = {}


def build_nc(variant=VARIANT):
    """Build the SPMD Bass program (identical on all 8 cores)."""
    nc = bacc.Bacc("TRN2", target_bir_lowering=False, debug=False, num_devices=8)
    xT = nc.dram_tensor("xT", [DIM, N], BF16, kind="ExternalInput").ap()
    # [qm0 | km0 | v | qm1 | km1] so a small early DMA unblocks the first phase
    w3 = nc.dram_tensor("w3", [DIM, 3 * HL * DIM_HEAD], BF16, kind="ExternalInput").ap()
    # head-pair stacked rows: wo2[s*64+d, hp, :] = w_out[(2hp+s)*64+d, :]
    wo = nc.dram_tensor("wo", [P, 2, DIM], BF16, kind="ExternalInput").ap()
    # exp(spd) in bf16: [hp, ib, jj, jb, s, ii]
    spdT = nc.dram_tensor("spdT", [2, 2, P, 8, 2, 512], BF16, kind="ExternalInput").ap()
    y = nc.dram_tensor("y", [N, DIM], F32, kind="ExternalOutput").ap()

    from contextlib import ExitStack

    with tile.TileContext(nc) as tc, ExitStack() as ctx:
        const = ctx.enter_context(tc.tile_pool(name="const", bufs=1))
        sb = ctx.enter_context(tc.tile_pool(name="sb", bufs=1))
        spd_pool = ctx.enter_context(tc.tile_pool(name="spd", bufs=3))
        ex_pool = ctx.enter_context(tc.tile_pool(name="ex", bufs=5))
        pr_pool = ctx.enter_context(tc.tile_pool(name="pr", bufs=6))
        nrm_pool = ctx.enter_context(tc.tile_pool(name="nrm", bufs=2))
        ps = ctx.enter_context(tc.tile_pool(name="ps", bufs=2, space="PSUM"))
        ps4 = ctx.enter_context(tc.tile_pool(name="ps4", bufs=4, space="PSUM"))

        # ---- resident loads -------------------------------------------------
        # sync queue: w_qkv m0 cols, xT in K-chunks, v cols, per-phase spd
        xT_sb = sb.tile([P, 4, N], BF16)
        w3_sb = sb.tile([P, 4, 768], BF16, tag="w3")
        xT_r = xT.rearrange("(kb p) n -> p kb n", p=P)
        w3_r = w3.rearrange("(kb p) m -> p kb m", p=P)
        nc.sync.dma_start(w3_sb[:, :, 0:256], w3_r[:, :, 0:256])      # q/k m0
        for kb in range(4):
            nc.sync.dma_start(xT_sb[:, kb, :], xT_r[:, kb, :])
        nc.sync.dma_start(w3_sb[:, :, 256:512], w3_r[:, :, 256:512])  # v
        # one queue, dependency order: the DMA engines serve queue heads
        # fairly, so anything on a second queue would steal bandwidth from
        # the prologue-critical stream above
        st00 = spd_pool.tile([P, 8, 2, 512], BF16, tag="spd", name="spd_0_0")
        nc.sync.dma_start(st00[:, 0:4], spdT[0, 0, :, 0:4])
        nc.sync.dma_start(st00[:, 4:8], spdT[0, 0, :, 4:8])
        nc.sync.dma_start(w3_sb[:, :, 512:768], w3_r[:, :, 512:768])  # q/k m1
        wo_sb = sb.tile([P, 2, DIM], BF16, tag="wo")
        nc.sync.dma_start(wo_sb[:], wo[:])

        # PE p-state warm-up + ACT exp-table preload during the initial DMA
        # wait: the ramp clock starts at the first matmul and reaches full
        # rate 3us later, so start it as early as possible — a Pool memset
        # (not DVE, whose queue is behind other init work) feeds the first
        # dummy matmul at ~0.3us.
        wdat = const.tile([65, 512], BF16, tag="wdat")
        nc.vector.memset(wdat[:], 1.0)
        warm = ps.tile([P, 1024], F32, tag="big", name="warm")
        for w in range(2):
            nc.tensor.matmul(warm[0:64, 0:512], wdat[64:65, 0:64], wdat[64:65, :],
                             start=True, stop=True)
        warm_ex = const.tile([1, 8], BF16, tag="warm_ex")
        nc.scalar.activation(warm_ex[:], wdat[64:65, 0:8], EXP)

        ones32 = const.tile([P, 1], F32)
        nc.vector.memset(ones32[:], 1.0)
        # ones row at partition 64: lhsT of the K=1 rowsum-reciprocal
        # broadcast matmul (both operands at partition 64 — HW-exact)
        ones65f = const.tile([65, DIM_HEAD], F32, tag="ones65f")
        nc.vector.memset(ones65f[:], 1.0)
        ones65 = const.tile([65, DIM_HEAD], F32R, tag="ones65")
        nc.vector.tensor_copy(ones65[:], ones65f[:])

        # ---- qkv projections ------------------------------------------------
        qT_sb = sb.tile([P, 2, N], F32R, tag="qT")
        kT_sb = sb.tile([P, 2, N], F32R, tag="kT")
        v_aug = sb.tile([P, 8, HL * 65], BF16, tag="vaug")
        v_cols = v_aug[:].rearrange("p jb (h c) -> p jb h c", c=65)
        nc.vector.tensor_copy(
            v_cols[:, :, :, 64:65],
            ones32[:, None, :, None].to_broadcast((P, 8, HL, 1)),
        )

        # m0 q and k interleaved per K-chunk so the matmuls pipeline behind
        # the xT chunk DMAs; copies split in halves on ACT — the ib=0 halves
        # (cols 0:512) unblock the first dots, the others only gate jb4+
        pq0 = ps.tile([P, 1024], F32, tag="big", name="pq0")
        pk0 = ps.tile([P, 1024], F32, tag="big", name="pk0")
        for kb in range(4):
            for qk, pq in ((0, pq0), (1, pk0)):
                for nb in range(2):
                    nc.tensor.matmul(
                        pq[:, nb * 512:(nb + 1) * 512],
                        w3_sb[:, kb, qk * 128:qk * 128 + 128],
                        xT_sb[:, kb, nb * 512:(nb + 1) * 512],
                        start=(kb == 0),
                        stop=(kb == 3),
                    )
        # q on ACT, k on DVE: the ib=0 halves run in parallel and unblock
        # the first dots; the late halves must not trail into phase 0 or
        # the pd ring would wait on them behind exp
        nc.scalar.copy(qT_sb[:, 0, 0:512], pq0[:, 0:512])
        nc.vector.tensor_copy(kT_sb[:, 0, 0:512], pk0[:, 0:512])
        nc.scalar.copy(qT_sb[:, 0, 512:1024], pq0[:, 512:1024])
        nc.vector.tensor_copy(kT_sb[:, 0, 512:1024], pk0[:, 512:1024])

        def v_proj():
            # v projection: narrow tiles on the 4-slot ring; emitted INSIDE
            # phase 0 after its jb0-3 dots so the exp stream starts ~2us
            # earlier (v is only needed by attn@v, a phase-length later)
            for jb in range(8):
                pv = ps4.tile([P, 512], F32, tag="po", name=f"pv_{jb}")
                for kb in range(4):
                    nc.tensor.matmul(
                        pv[:, :256],
                        xT_sb[:, kb, jb * 128:(jb + 1) * 128],
                        w3_sb[:, kb, 256:512],
                        start=(kb == 0),
                        stop=(kb == 3),
                    )
                nc.vector.tensor_copy(
                    v_cols[:, jb, :, :64],
                    pv[:, :256].rearrange("p (h c) -> p h c", c=64),
                )

        # ---- attention ------------------------------------------------------
        # normalized attention output, transposed, head-pair stacked:
        # scaled[s*64+d, hp, i] (128 partitions -> K=128 proj matmuls)
        scaled = sb.tile([P, 2, N], BF16, tag="scaled")
        y_all = sb.tile([P, 8, 512], F32, tag="yall")

        def do_norm(po, hp, ib, halves=1, flush=False):
            # head-pair normalization: 1/rowsums (DVE) -> the exp'd scores
            # copied out of PSUM head-pair-stacked (DVE can only read ONE
            # PSUM operand per op, so the rescale multiplicand must be in
            # SBUF; the copy also frees the po ring slots early) -> K=1
            # broadcast matmuls (s=1 written at partition base 64) ->
            # rescale into `scaled`. halves=2 splits the rescale along i so
            # the flush projections start earlier; flush=True borrows the
            # then-idle ACT engine for the PSUM copies.
            rc = nrm_pool.tile([65, 1024], F32R, tag="rc", name=f"rc_{hp}_{ib}")
            with nc.allow_low_precision(reason="f32r recip is plenty for softmax denom"):
                for s in range(2):
                    nc.vector.reciprocal(rc[64:65, s * 512:(s + 1) * 512],
                                         po[s][64:65, :])
            pocp = nrm_pool.tile([P, 512], BF16, tag="pocp", name=f"pocp_{hp}_{ib}")
            pbs = []
            for s in range(2):
                cp = nc.scalar.copy if flush else nc.vector.tensor_copy
                cp(pocp[s * 64:(s + 1) * 64, :], po[s][0:64, :])
                pb = ps4.tile([P, 512], F32, tag="po", name=f"pb_{hp}_{ib}_{s}")
                nc.tensor.matmul(pb[0:64, :], ones65[64:65, :],
                                 rc[64:65, s * 512:(s + 1) * 512],
                                 start=True, stop=True)
                pbs.append(pb)
            w = 512 // halves
            for h in range(halves):
                for s in range(2):
                    nc.vector.tensor_tensor(
                        scaled[s * 64:(s + 1) * 64, hp,
                               ib * 512 + h * w:ib * 512 + (h + 1) * w],
                        pocp[s * 64:(s + 1) * 64, h * w:(h + 1) * w],
                        pbs[s][0:64, h * w:(h + 1) * w],
                        MULT,
                    )

        def proj(io, copy_eng="dve", dma_eng="sync"):
            # one 128-row block of y: K=128 over each head pair, copy out,
            # stream to DRAM (engines chosen by the caller for tail overlap)
            py = ps4.tile([P, 512], F32, tag="po", name=f"py_{io}")
            for hp in range(2):
                nc.tensor.matmul(
                    py[:],
                    scaled[:, hp, io * 128:(io + 1) * 128],
                    wo_sb[:, hp, :],
                    start=(hp == 0),
                    stop=(hp == 1),
                )
            if copy_eng == "act":
                nc.scalar.copy(y_all[:, io, :], py[:])
            else:
                nc.vector.tensor_copy(y_all[:, io, :], py[:])
            getattr(nc, dma_eng).dma_start(
                y[io * 128:(io + 1) * 128, :], y_all[:, io, :])

        def m1_half(qk, nb, dst):
            # one [128, 512] quarter of the m1 q/k projection: 4 matmuls +
            # a DVE copy, small enough to slot between two dots pairs
            wofs = 512 + qk * 128
            pq1 = ps4.tile([P, 512], F32, tag="po", name=f"pq1_{qk}_{nb}")
            for kb in range(4):
                nc.tensor.matmul(
                    pq1[:],
                    w3_sb[:, kb, wofs:wofs + 128],
                    xT_sb[:, kb, nb * 512:(nb + 1) * 512],
                    start=(kb == 0),
                    stop=(kb == 3),
                )
            nc.vector.tensor_copy(
                dst[:, 1, nb * 512:(nb + 1) * 512], pq1[:])

        # ---- software-pipelined phase loop ----------------------------------
        # ACT paces the steady state (8 back-to-back 1us exps per phase); all
        # other work is scheduled around keeping its pd ping-pong fed:
        #   * dots for jb0..3 are issued ahead of everything else at a phase
        #     boundary (the next exp must never wait),
        #   * the late attn@v matmuls of a phase (jb 3, 6, 7 — Pool's jb3
        #     multiply and the last DVE multiplies finish near the boundary)
        #     are CARRIED into the next phase and issued s-major with the
        #     accumulation stop flags so the rowsum reciprocals start early,
        #   * the previous phase's normalization (recips -> K=1 broadcasts ->
        #     partition-shifted rescales) then runs in the next phase's PE/DVE
        #     slack, freeing its po slots just before attn@v jb0 needs them,
        #   * jb2/jb3 multiplies run on Pool (early, so their 2.2us latency
        #     hides inside the phase), the rest on DVE.
        def dots_exp_mult(hp, ib, jb, st, split_s=False, pool=False):
            pd = ps.tile([P, 1024], F32, tag="big", name=f"pd_{hp}_{ib}_{jb}")
            for s in range(2):
                nc.tensor.matmul(
                    pd[:, s * 512:(s + 1) * 512],
                    kT_sb[64 * s:64 * s + 64, hp, jb * 128:(jb + 1) * 128],
                    qT_sb[64 * s:64 * s + 64, hp, ib * 512:(ib + 1) * 512],
                    start=True,
                    stop=True,
                )
            ex = ex_pool.tile([P, 1024], BF16, tag="ex", name=f"ex_{hp}_{ib}_{jb}")
            pr = pr_pool.tile([P, 1024], BF16, tag="pr", name=f"pr_{hp}_{ib}_{jb}")
            eng = nc.gpsimd if pool else nc.vector
            if split_s:
                # per-s exp/multiply so head s=0's rowsum (and reciprocal)
                # is ready a half-tile earlier in the flush
                for s in range(2):
                    nc.scalar.activation(ex[:, s * 512:(s + 1) * 512],
                                         pd[:, s * 512:(s + 1) * 512], EXP)
                    eng.tensor_tensor(
                        pr[:, s * 512:(s + 1) * 512],
                        ex[:, s * 512:(s + 1) * 512],
                        st[:, jb, s],
                        MULT,
                    )
            else:
                nc.scalar.activation(ex[:], pd[:], EXP)
                eng.tensor_tensor(
                    pr[:], ex[:],
                    st[:, jb].rearrange("p s i -> p (s i)"),
                    MULT,
                )
            return pr

        def attnv(po, hp, jb, pr, start=False, stop=False, s_only=None):
            for s in range(2) if s_only is None else (s_only,):
                h = 2 * hp + s
                nc.tensor.matmul(
                    po[s][0:65, :],
                    v_aug[:, jb, h * 65:(h + 1) * 65],
                    pr[:, s * 512:(s + 1) * 512],
                    start=start,
                    stop=stop,
                )

        def carry_s(po, hp, carried, s):
            # one head's carried attn@v chain; the stop-flag matmul issues
            # as early as possible so its rowsum reciprocal overlaps the
            # other head's chain
            for n, (jb, pr) in enumerate(carried):
                attnv(po, hp, jb, pr, stop=(n == len(carried) - 1), s_only=s)

        prev = None      # (po, hp, ib, carried) of the previous phase
        for ib in range(2):          # i block of 512 (outer: frees proj early)
            for hp in range(2):      # head pair (local heads 2hp, 2hp+1)
                ph0 = (ib == 0 and hp == 0)
                last = (ib == 1 and hp == 1)
                if ph0:
                    st = st00
                else:
                    st = spd_pool.tile([P, 8, 2, 512], BF16, tag="spd",
                                       name=f"spd_{hp}_{ib}")
                    nc.sync.dma_start(st[:, 0:4], spdT[hp, ib, :, 0:4])
                    nc.sync.dma_start(st[:, 4:8], spdT[hp, ib, :, 4:8])
                pool_jbs = (2, 3)
                prs = {}
                prs[0] = dots_exp_mult(hp, ib, 0, st)
                prs[1] = dots_exp_mult(hp, ib, 1, st)
                # previous phase's late attn@v + normalization, split around
                # this phase's jb2/jb3 dots so the exp stream never starves
                if prev is not None:
                    carry_s(prev[0], prev[1], prev[3], 0)
                prs[2] = dots_exp_mult(hp, ib, 2, st, pool=2 in pool_jbs)
                if prev is not None:
                    carry_s(prev[0], prev[1], prev[3], 1)
                prs[3] = dots_exp_mult(hp, ib, 3, st, pool=3 in pool_jbs)
                if ib == 1 and hp == 0:
                    # qT m1 second half: needed by the (ib1, hp1) phase
                    m1_half(0, 1, qT_sb)
                if prev is not None:
                    do_norm(prev[0], prev[1], prev[2])
                if ph0:
                    # v projection rides the early-phase pd-ring slack (only
                    # attn@v consumes it, nearly a phase later)
                    v_proj()
                prs[4] = dots_exp_mult(hp, ib, 4, st, pool=4 in pool_jbs)
                if ph0:
                    m1_half(0, 0, qT_sb)
                prs[5] = dots_exp_mult(hp, ib, 5, st)
                if ph0:
                    m1_half(1, 0, kT_sb)
                if not ph0:
                    po = [ps4.tile([128, 512], F32, tag="po",
                                   name=f"po_{hp}_{ib}_{s}") for s in range(2)]
                    attnv(po, hp, 0, prs[0], start=True)
                    attnv(po, hp, 1, prs[1])
                prs[6] = dots_exp_mult(hp, ib, 6, st)
                prs[7] = dots_exp_mult(hp, ib, 7, st, split_s=last)
                if not ph0:
                    attnv(po, hp, 2, prs[2])
                    attnv(po, hp, 4, prs[4])
                if ph0:
                    # kT m1 second half at the very end of phase 0: its PE
                    # slot is free here and its DVE copy completes well
                    # before phase 1's jb4 dots need it
                    m1_half(1, 1, kT_sb)
                    # po allocated only after the m1 tiles: a ps4-ring slot
                    # reuse of po by an m1 tile would deadlock phase 1's dots
                    # against phase 1's own normalization
                    po = [ps4.tile([128, 512], F32, tag="po",
                                   name=f"po_{hp}_{ib}_{s}") for s in range(2)]
                    attnv(po, hp, 0, prs[0], start=True)
                    attnv(po, hp, 1, prs[1])
                else:
                    attnv(po, hp, 5, prs[5])
                if ib == 1 and not last:
                    proj(2 * hp + 0)
                    proj(2 * hp + 1)
                if ib == 1 and last:
                    # last phase: both projections after the final multiply,
                    # copies on the then-idle ACT
                    proj(2 * hp + 0, "act")
                    proj(2 * hp + 1, "act")
                carry = [2, 4, 5, 3, 6, 7] if ph0 else [3, 6, 7]
                prev = (po, hp, ib, [(jb, prs[jb]) for jb in carry])

        # flush: last phase's late attn@v + normalization + remaining
        # projections, spread across engines and all three DMA queues
        p_po, p_hp, p_ib, p_carried = prev
        carry_s(p_po, p_hp, p_carried, 0)
        carry_s(p_po, p_hp, p_carried, 1)
        do_norm(p_po, p_hp, p_ib, halves=2, flush=True)
        proj(4, "act", "gpsimd")
        proj(5, "dve", "sync")
        proj(6, "act", "scalar")
        proj(7, "dve", "gpsimd")

    nc.compile()
    return nc


def _get_nc(variant=VARIANT):
    if variant not in _NC:
        _NC[variant] = build_nc(variant)
    return _NC[variant]


def make_in_maps(x, spd, head_keep, w_qkv, w_out, variant=VARIANT):
    x = np.asarray(x, np.float32)
    spd = np.asarray(spd, np.float32)
    keep = np.asarray(head_keep, np.float32)
    w_qkv = np.asarray(w_qkv, np.float32)
    w_out = np.asarray(w_out, np.float32)
    cfac = keep * (HEADS / keep.sum())

    in_maps = []
    for c in range(8):
        bi, hh = divmod(c, 2)
        h0 = hh * HL
        hs = slice(h0 * DIM_HEAD, (h0 + HL) * DIM_HEAD)
        xT = np.ascontiguousarray(x[bi].T).astype(ml_dtypes.bfloat16)
        q_cols = w_qkv[:, hs] * np.float32(SCALE)
        k_cols = w_qkv[:, DIM + h0 * DIM_HEAD:DIM + (h0 + HL) * DIM_HEAD]
        v_cols_h = w_qkv[:, 2 * DIM + h0 * DIM_HEAD:2 * DIM + (h0 + HL) * DIM_HEAD]
        w3 = np.ascontiguousarray(np.concatenate(
            [q_cols[:, :128], k_cols[:, :128], v_cols_h,
             q_cols[:, 128:], k_cols[:, 128:]],
            axis=1,
        )).astype(ml_dtypes.bfloat16)
        wo_rows = w_out[hs, :] * np.repeat(cfac[h0:h0 + HL], DIM_HEAD)[:, None]
        # [h, d, dim] -> [hp, s, d, dim] -> [(s d), hp, dim]
        wo2 = wo_rows.reshape(2, 2, DIM_HEAD, DIM).transpose(1, 2, 0, 3).reshape(
            P, 2, DIM)
        wo2 = np.ascontiguousarray(wo2).astype(ml_dtypes.bfloat16)
        sp = spd[bi, h0:h0 + HL]  # [HL, i, j] with h = 2*hp + s
        # [hp, s, ib, ii, jb, jj] -> [hp, ib, jj, jb, s, ii]
        spdT = sp.reshape(2, 2, 2, 512, 8, 128).transpose(0, 2, 5, 4, 1, 3)
        spdT = np.exp(spdT).astype(ml_dtypes.bfloat16)
        in_maps.append({"xT": xT, "w3": w3, "wo": wo2,
                        "spdT": np.ascontiguousarray(spdT)})
    return in_maps


def kernel(x, spd, head_keep, w_qkv, w_out, b_out):
    assert x.shape == (B, N, DIM) and spd.shape == (B, HEADS, N, N)
    nc = _get_nc()
    in_maps = make_in_maps(x, spd, head_keep, w_qkv, w_out)
    res = run_bass_kernel_spmd(nc, in_maps, core_ids=list(range(8)))
    out = np.empty((B, N, DIM), np.float32)
    for bi in range(B):
        out[bi] = res.results[2 * bi]["y"] + res.results[2 * bi + 1]["y"]
    out += np.asarray(b_out, np.float32)[None, None, :]
    return out
